# revision 8
# baseline (speedup 1.0000x reference)
"""GCN VGAE encoder (nn_Encoder_25065429139538) on 8 Trainium2 NeuronCores.

Strategy (sharding_hint: shard nodes across cores, partition edges by dst,
replicate weights):
  - Nodes padded to 100352 = 8 x 12544; core d owns dst rows [d*SH, (d+1)*SH).
  - Per-edge normalization coefficients (symmetric GCN norm, incl. self-loops)
    are folded into a single per-edge scalar on the host; duplicate (src,dst)
    pairs are merged. Edges are grouped per core by dst 128-row block and by
    src quarter (so dma_gather's int16 indices address a <32768-row subtable).
  - Slot packing is continuous per src-quarter stream: per (block, quarter)
    cell sizes are the max over cores (SPMD shared structure), but cells are
    NOT rounded to 128; a 128-slot chunk may span two adjacent dst blocks and
    then contributes to both via a second one-hot matmul compared against
    iota+128*k.
  - Aggregation commutes with the dense projections, so each layer gathers raw
    table rows h[src] (dma_gather, 4 SWDGE queues), scales them by the edge
    coefficient (DVE), and reduces segments with a one-hot matmul on the
    TensorEngine accumulating in PSUM -- no scatter DMA at all.
  - The projection W runs per 128-row dst block right after its PSUM segment
    sum resolves; h is AllGather'd between layers to rebuild the full gather
    table. mu/logstd share one aggregation pass.
"""

import math

import numpy as np

import concourse.bass as bass
import concourse.bacc as bacc
import concourse.mybir as mybir
import concourse.tile as tile
from concourse.bass_utils import run_bass_kernel_spmd
from concourse.library_config import mlp

# ---- problem constants (hardcoded per contract) ----
N = 100000
FIN, HID, OUT = 128, 64, 32
NCORES = 8

# ---- layout constants ----
SH = 12544            # rows per core (100352 / 8)
NPAD = SH * NCORES    # 100352
NBLK = SH // 128      # 98 dst blocks per core
NSUB = 4              # src subtables (int16 gather indices)
SUB = NPAD // NSUB    # 25088 rows per subtable
SLOTS = 1024          # gather slots per dma_gather instruction (SWDGE ring cap)
CPG = SLOTS // 128    # chunks per gather group = 8
QBLK = [25, 25, 24, 24]            # dst-block chunking for dense loads/stores
QOFF = [0, 25, 50, 74]


def _wrap_idx(slots_i16):
    """[G*SLOTS] int16 -> [G, 128, SLOTS//16]: slot i at [i%16 (+16m), i//16]."""
    g = slots_i16.reshape(-1, SLOTS // 16, 16)          # [G, S/16, 16]
    g = np.swapaxes(g, 1, 2)                            # [G, 16, S/16]
    return np.tile(g, (1, 8, 1)).astype(np.int16)       # [G, 128, S/16]


def _prep(x, edge_index, edge_weight):
    """Host-side edge partitioning. Returns (plan, per-core arrays)."""
    src = np.asarray(edge_index[0], dtype=np.int64)
    dst = np.asarray(edge_index[1], dtype=np.int64)
    ew = np.asarray(edge_weight, dtype=np.float32)

    deg_w = np.zeros(N, np.float32)
    np.add.at(deg_w, dst, ew)
    deg_w += 1.0  # self-loop weight
    deg_1 = (np.bincount(dst, minlength=N) + 1).astype(np.float32)
    dinv_w = 1.0 / np.sqrt(deg_w)
    dinv_1 = 1.0 / np.sqrt(deg_1)

    nw = dinv_w[src] * ew * dinv_w[dst]
    n1 = dinv_1[src] * dinv_1[dst]

    # merge duplicate (src, dst) pairs (self-loops handled separately)
    key = src * NPAD + dst
    ukey, inv = np.unique(key, return_inverse=True)
    unw = np.zeros(len(ukey), np.float32)
    un1 = np.zeros(len(ukey), np.float32)
    np.add.at(unw, inv, nw)
    np.add.at(un1, inv, n1)
    usrc = ukey // NPAD
    udst = ukey % NPAD

    core = udst // SH
    t_all = (udst % SH) // 128
    dloc_all = (udst % SH) % 128
    s_all = usrc // SUB
    sloc_all = usrc % SUB

    # shared chunk structure: per-(t, s) slot count = max over cores
    cell = (core * NBLK + t_all) * NSUB + s_all
    cnt = np.bincount(cell, minlength=NCORES * NBLK * NSUB).reshape(NCORES, NBLK, NSUB)
    shared_cnt = cnt.max(axis=0)                      # [NBLK, NSUB]
    cum = np.zeros((NBLK + 1, NSUB), np.int64)
    cum[1:] = np.cumsum(shared_cnt, axis=0)           # exclusive prefix per s
    total_s = cum[NBLK]                               # slots per stream
    chunks_s = [int(math.ceil(int(c) / 128)) for c in total_s]
    G_s = [int(math.ceil(c / CPG)) for c in chunks_s]

    # chunk -> base block (block containing the chunk's first slot), per s
    t_base = []
    relmax = 0
    for s in range(NSUB):
        tb = np.searchsorted(cum[:, s], np.arange(chunks_s[s]) * 128, side="right") - 1
        t_base.append(tb)
        # per-block chunk ranges and rel offsets
        for t in range(NBLK):
            if shared_cnt[t, s] == 0:
                continue
            c_lo = int(cum[t, s] // 128)
            c_hi = int((cum[t + 1, s] - 1) // 128)
            relmax = max(relmax, t - int(tb[c_lo]))
    R = relmax + 1
    assert R <= 4, f"rel span {R} too large"

    # per-slot shared rel: rel of the block owning slot p vs its chunk's base
    rel_p = []
    for s in range(NSUB):
        nslot = chunks_s[s] * 128
        owner = np.searchsorted(cum[:, s], np.arange(nslot), side="right") - 1
        owner = np.minimum(owner, NBLK - 1)  # pad tail slots -> last block
        rp = owner - t_base[s][np.arange(nslot) // 128]
        rel_p.append(rp.astype(np.int64))

    per_core = []
    order = np.lexsort((sloc_all, t_all, s_all, core))
    osloc, os_, ot, odloc, onw, on1, ocore = (
        sloc_all[order], s_all[order], t_all[order],
        dloc_all[order], unw[order], un1[order], core[order],
    )
    cstart = np.searchsorted(ocore, np.arange(NCORES + 1))
    for d in range(NCORES):
        lo, hi = cstart[d], cstart[d + 1]
        dt, ds = ot[lo:hi], os_[lo:hi]
        dsl, ddl = osloc[lo:hi], odloc[lo:hi]
        dnw, dn1 = onw[lo:hi], on1[lo:hi]
        srcw, dlocv, nwv, n1v = [], [], [], []
        cell_d = ds * NBLK + dt
        cello = np.searchsorted(cell_d, np.arange(NSUB * NBLK + 1))
        for s in range(NSUB):
            nslot = chunks_s[s] * 128
            sl = np.zeros(nslot, np.int64)
            dl = np.zeros(nslot, np.float32)
            wv = np.zeros(nslot, np.float32)
            v1 = np.zeros(nslot, np.float32)
            for t in range(NBLK):
                a, b = cello[s * NBLK + t], cello[s * NBLK + t + 1]
                n = b - a
                p0 = int(cum[t, s])
                sl[p0:p0 + n] = dsl[a:b]
                dl[p0:p0 + n] = ddl[a:b]
                wv[p0:p0 + n] = dnw[a:b]
                v1[p0:p0 + n] = dn1[a:b]
            dl += 128.0 * rel_p[s]  # fold chunk-relative block offset
            gpad = G_s[s] * SLOTS
            slp = np.zeros(gpad, np.int64)
            dlp = np.zeros(gpad, np.float16)
            wvp = np.zeros(gpad, np.float32)
            v1p = np.zeros(gpad, np.float32)
            slp[:nslot] = sl
            dlp[:nslot] = dl.astype(np.float16)
            wvp[:nslot] = wv
            v1p[:nslot] = v1
            srcw.append(_wrap_idx(slp.astype(np.int16)))
            dlocv.append(dlp.reshape(G_s[s], CPG, 128).transpose(0, 2, 1).copy())
            nwv.append(wvp.reshape(G_s[s], CPG, 128).transpose(0, 2, 1).copy())
            n1v.append(v1p.reshape(G_s[s], CPG, 128).transpose(0, 2, 1).copy())
        # self-loop coeffs: [128, NBLK], value at [v, t] = dinv^2 of node
        v_glob = np.arange(d * SH, d * SH + SH, dtype=np.int64)
        swv = np.zeros(SH, np.float32)
        sv1 = np.zeros(SH, np.float32)
        real = v_glob < N
        swv[real] = (dinv_w * dinv_w)[v_glob[real]]
        sv1[real] = (dinv_1 * dinv_1)[v_glob[real]]
        snwv = swv.reshape(NBLK, 128).T.copy()
        sn1v = sv1.reshape(NBLK, 128).T.copy()
        per_core.append((srcw, dlocv, nwv, n1v, snwv, sn1v))

    plan = dict(shared_cnt=shared_cnt, cum=cum, chunks_s=chunks_s, G_s=G_s,
                t_base=t_base, R=R)
    return plan, per_core


def _build(plan):
    f32 = mybir.dt.float32
    f16 = mybir.dt.float16
    cum, chunks_s, G_s, t_base, R = (
        plan["cum"], plan["chunks_s"], plan["G_s"], plan["t_base"], plan["R"])
    nc = bacc.Bacc(None, target_bir_lowering=False, num_swdge_queues=4,
                   num_devices=NCORES, dynamic_dma_scratch_size=49152)

    xs_d = nc.dram_tensor("xs", [SH, FIN], f32, kind="ExternalInput")
    srcw_d = [nc.dram_tensor(f"srcw{s}", [G_s[s], 128, SLOTS // 16], mybir.dt.int16, kind="ExternalInput") for s in range(NSUB)]
    dloc_d = [nc.dram_tensor(f"dloc{s}", [G_s[s], 128, CPG], f16, kind="ExternalInput") for s in range(NSUB)]
    nw_d = [nc.dram_tensor(f"nw{s}", [G_s[s], 128, CPG], f32, kind="ExternalInput") for s in range(NSUB)]
    n1_d = [nc.dram_tensor(f"n1{s}", [G_s[s], 128, CPG], f32, kind="ExternalInput") for s in range(NSUB)]
    snw_d = nc.dram_tensor("snw", [128, NBLK], f32, kind="ExternalInput")
    sn1_d = nc.dram_tensor("sn1", [128, NBLK], f32, kind="ExternalInput")
    w1_d = nc.dram_tensor("W1", [FIN, HID], f32, kind="ExternalInput")
    w2_d = nc.dram_tensor("W2", [HID, HID], f32, kind="ExternalInput")
    wm_d = nc.dram_tensor("Wmu", [HID, OUT], f32, kind="ExternalInput")
    wl_d = nc.dram_tensor("Wls", [HID, OUT], f32, kind="ExternalInput")
    b1_d = nc.dram_tensor("b1", [HID, 1], f32, kind="ExternalInput")
    b2_d = nc.dram_tensor("b2", [HID, 1], f32, kind="ExternalInput")
    bm_d = nc.dram_tensor("bmu", [OUT, 1], f32, kind="ExternalInput")
    bl_d = nc.dram_tensor("bls", [OUT, 1], f32, kind="ExternalInput")
    iotaf_d = nc.dram_tensor("iotaf", [128, CPG * 128], f16, kind="ExternalInput")
    iotar_d = nc.dram_tensor("iotar", [128, R * 128], f16, kind="ExternalInput")
    ident_d = nc.dram_tensor("ident", [128, 128], f32, kind="ExternalInput")
    identh_d = nc.dram_tensor("identh", [128, 128], f16, kind="ExternalInput")
    out_d = nc.dram_tensor("out", [SH, 2 * OUT], f32, kind="ExternalOutput")

    ag_in = [nc.dram_tensor(f"ag_in{i}", [SH, HID], f32) for i in range(3)]
    tables = [nc.dram_tensor(f"table{i}", [NPAD, HID], f32, addr_space="Shared") for i in range(3)]

    with tile.TileContext(nc) as tc:
        with (
            tc.tile_pool(name="const", bufs=1) as kpool,
            tc.tile_pool(name="idx", bufs=2) as ipool,
            tc.tile_pool(name="meta", bufs=2) as mpool,
            tc.tile_pool(name="g", bufs=3) as gpool,
            tc.tile_pool(name="b", bufs=2) as bpool,
            tc.tile_pool(name="b2", bufs=4) as b2pool,
            tc.tile_pool(name="selfd", bufs=2) as selfpool,
            tc.tile_pool(name="stage", bufs=2) as spool,
            tc.tile_pool(name="tmp", bufs=4) as tpool,
            tc.tile_pool(name="pagg", bufs=2, space="PSUM") as pagg,
            tc.tile_pool(name="pmm", bufs=2, space="PSUM") as pmm,
            tc.tile_pool(name="ptr", bufs=2, space="PSUM") as ptr,
        ):
            nc.gpsimd.load_library(mlp)

            iotaf_t = kpool.tile([128, CPG * 128], f16)
            nc.sync.dma_start(iotaf_t[:], iotaf_d[:])
            iotar_t = kpool.tile([128, R * 128], f16)
            nc.sync.dma_start(iotar_t[:], iotar_d[:])
            ident_t = kpool.tile([128, 128], f32)
            nc.sync.dma_start(ident_t[:], ident_d[:])
            identh_t = kpool.tile([128, 128], f16)
            nc.sync.dma_start(identh_t[:], identh_d[:])
            w1_t = kpool.tile([FIN, HID], f32)
            nc.sync.dma_start(w1_t[:], w1_d[:])
            w2_t = kpool.tile([HID, HID], f32)
            nc.sync.dma_start(w2_t[:], w2_d[:])
            wm_t = kpool.tile([HID, OUT], f32)
            nc.sync.dma_start(wm_t[:], wm_d[:])
            wl_t = kpool.tile([HID, OUT], f32)
            nc.sync.dma_start(wl_t[:], wl_d[:])
            b1_t = kpool.tile([HID, 1], f32)
            nc.sync.dma_start(b1_t[:], b1_d[:])
            b2_t = kpool.tile([HID, 1], f32)
            nc.sync.dma_start(b2_t[:], b2_d[:])
            bm_t = kpool.tile([OUT, 1], f32)
            nc.sync.dma_start(bm_t[:], bm_d[:])
            bl_t = kpool.tile([OUT, 1], f32)
            nc.sync.dma_start(bl_t[:], bl_d[:])
            snw_t = kpool.tile([128, NBLK], f32)
            nc.sync.dma_start(snw_t[:], snw_d[:])
            sn1_t = kpool.tile([128, NBLK], f32)
            nc.sync.dma_start(sn1_t[:], sn1_d[:])

            def aggregate_project(table_i, norm_of_s, snorm_t, store_block):
                """Per dst block: PSUM segment-sum then store_block(t, ps)."""
                F = HID
                cur = {}
                gq = [0]
                dsh_q = [None]

                def load_self(q):
                    t0, nb = QOFF[q], QBLK[q]
                    ds = selfpool.tile([128, max(QBLK), F], f32, tag="selfd")
                    nc.sync.dma_start(
                        ds[:, :nb, :],
                        ag_in[table_i][t0 * 128:(t0 + nb) * 128, :].rearrange(
                            "(t p) f -> p t f", p=128),
                    )
                    dsh = selfpool.tile([128, max(QBLK), F], f16, tag="selfdh")
                    nc.vector.tensor_tensor(
                        out=dsh[:, :nb, :],
                        in0=ds[:, :nb, :],
                        in1=snorm_t[:, t0:t0 + nb].to_broadcast([128, nb, F]),
                        op=mybir.AluOpType.mult,
                    )
                    dsh_q[0] = (q, dsh)

                def ensure(s, g):
                    if s in cur and cur[s][0] == g:
                        return cur[s][1]
                    nch = min(CPG, chunks_s[s] - g * CPG)
                    ni = nch * 128
                    it = ipool.tile([128, SLOTS // 16], mybir.dt.int16, tag="idx")
                    nc.sync.dma_start(it[:], srcw_d[s][g])
                    gt = gpool.tile([128, CPG, F], f32, tag=f"g{s}")
                    nc.gpsimd.dma_gather(
                        gt[:], tables[table_i][s * SUB:(s + 1) * SUB, :],
                        it[:], SLOTS, SLOTS, F, queue_num=gq[0] % 4,
                    )
                    gq[0] += 1
                    nt = mpool.tile([128, CPG], f32, tag=f"nm{s}")
                    nc.sync.dma_start(nt[:, :nch], norm_of_s(s)[g, :, :nch])
                    dt_ = mpool.tile([128, CPG], f16, tag=f"dl{s}")
                    nc.sync.dma_start(dt_[:, :nch], dloc_d[s][g, :, :nch])
                    bt = bpool.tile([128, CPG, 128], f16, tag=f"b{s}")
                    nc.vector.tensor_tensor(
                        out=bt[:, :nch, :],
                        in0=iotaf_t[:, :ni].rearrange("p (j v) -> p j v", j=nch),
                        in1=dt_[:, :nch].to_broadcast([128, nch, 128]),
                        op=mybir.AluOpType.is_equal,
                    )
                    gh = gpool.tile([128, CPG, F], f16, tag=f"gh{s}")
                    nc.vector.tensor_tensor(
                        out=gh[:, :nch, :],
                        in0=gt[:, :nch, :],
                        in1=nt[:, :nch].to_broadcast([128, nch, F]),
                        op=mybir.AluOpType.mult,
                    )
                    cur[s] = (g, (gh, bt, dt_))
                    return cur[s][1]

                for q in range(4):
                    load_self(q)
                    t0, nb = QOFF[q], QBLK[q]
                    for t in range(t0, t0 + nb):
                        ps = pagg.tile([128, 128], f32, tag="pagg")
                        # count matmuls for start/stop flags
                        nmm = 1
                        for s in range(NSUB):
                            if plan["shared_cnt"][t, s] == 0:
                                continue
                            c_lo = int(cum[t, s] // 128)
                            c_hi = int((cum[t + 1, s] - 1) // 128)
                            nmm += c_hi - c_lo + 1
                        nc.tensor.matmul(
                            ps[:F, :], lhsT=dsh_q[0][1][:, t - t0, :], rhs=identh_t[:],
                            start=True, stop=(nmm == 1),
                        )
                        ci = 1
                        for s in range(NSUB):
                            if plan["shared_cnt"][t, s] == 0:
                                continue
                            c_lo = int(cum[t, s] // 128)
                            c_hi = int((cum[t + 1, s] - 1) // 128)
                            for c in range(c_lo, c_hi + 1):
                                g, j = divmod(c, CPG)
                                gh, bt, dt_ = ensure(s, g)
                                rel = t - int(t_base[s][c])
                                if rel == 0:
                                    rhs = bt[:, j, :]
                                else:
                                    b2 = b2pool.tile([128, 1, 128], f16, tag="b2")
                                    nc.vector.tensor_tensor(
                                        out=b2[:],
                                        in0=iotar_t[:].rearrange(
                                            "p (r v) -> p r v", r=R)[:, rel:rel + 1, :],
                                        in1=dt_[:, j:j + 1].to_broadcast([128, 1, 128]),
                                        op=mybir.AluOpType.is_equal,
                                    )
                                    rhs = b2[:, 0, :]
                                nc.tensor.matmul(
                                    ps[:F, :], lhsT=gh[:, j, :], rhs=rhs,
                                    start=False, stop=(ci == nmm - 1),
                                )
                                ci += 1
                        store_block(t, ps)

            def make_store(w_t, b_t, func, dram_rows, width):
                """Returns store_block writing func(W.T@ps + b).T rows to DRAM."""
                state = {"st": None, "q": -1}

                def store_block(t, ps):
                    q = 0
                    while t >= QOFF[q] + QBLK[q]:
                        q += 1
                    t0, nb = QOFF[q], QBLK[q]
                    if state["q"] != q:
                        state["st"] = spool.tile([128, max(QBLK) * width], f32, tag="st", name="stq")
                        state["q"] = q
                    st = state["st"]
                    if w_t is None:
                        ht = tpool.tile([HID, 128], f32, tag="ht")
                        if func is None:
                            nc.vector.tensor_scalar_add(ht[:], ps[:HID, :], b_t[:])
                        else:
                            nc.scalar.activation(ht[:], ps[:HID, :], func, bias=b_t[:])
                    else:
                        aggb = tpool.tile([HID, 128], f32, tag="aggb")
                        nc.scalar.activation(aggb[:], ps[:HID, :], mybir.ActivationFunctionType.Copy)
                        ht = tpool.tile([width, 128], f32, tag="ht")
                        for k, (w, b) in enumerate(zip(w_t, b_t)):
                            wo = w.shape[1]
                            ph = pmm.tile([wo, 128], f32, tag="pmm")
                            nc.tensor.matmul(ph[:], lhsT=w[:], rhs=aggb[:], start=True, stop=True)
                            nc.vector.tensor_scalar_add(ht[k * wo:(k + 1) * wo, :], ph[:], b[:])
                    pt = ptr.tile([128, width], f32, tag="ptr")
                    nc.tensor.transpose(pt[:], ht[:width, :], ident_t[:width, :width])
                    nc.vector.tensor_copy(st[:, (t - t0) * width:(t - t0 + 1) * width], pt[:])
                    if t == t0 + nb - 1:
                        nc.sync.dma_start(
                            dram_rows[t0 * 128:(t0 + nb) * 128, :].rearrange(
                                "(t p) f -> p t f", p=128),
                            st[:, :nb * width].rearrange("p (t f) -> p t f", f=width),
                        )

                return store_block

            def allgather(i):
                nc.gpsimd.collective_compute(
                    "AllGather", mybir.AluOpType.bypass,
                    replica_groups=[list(range(NCORES))],
                    ins=[ag_in[i][:]], outs=[tables[i][:]],
                )

            # ---- pre-projection: table0 = x_shard @ W1 (no bias) ----
            st0 = {"st": None, "q": -1}
            for q in range(4):
                t0, nb = QOFF[q], QBLK[q]
                st = spool.tile([128, max(QBLK) * HID], f32, tag="st")
                for t in range(t0, t0 + nb):
                    xt = tpool.tile([128, FIN], f32, tag="xt")
                    nc.sync.dma_start(xt[:], xs_d[t * 128:(t + 1) * 128, :])
                    pxt = ptr.tile([128, 128], f32, tag="pxt")
                    nc.tensor.transpose(pxt[:], xt[:], ident_t[:])
                    xT = tpool.tile([128, 128], f32, tag="xT")
                    nc.vector.tensor_copy(xT[:], pxt[:])
                    ph0 = pmm.tile([HID, 128], f32, tag="pmm")
                    nc.tensor.matmul(ph0[:], lhsT=w1_t[:], rhs=xT[:], start=True, stop=True)
                    h0 = tpool.tile([HID, 128], f32, tag="ht")
                    nc.vector.tensor_copy(h0[:], ph0[:])
                    pt0 = ptr.tile([128, HID], f32, tag="ptr")
                    nc.tensor.transpose(pt0[:], h0[:], ident_t[:HID, :HID])
                    nc.vector.tensor_copy(st[:, (t - t0) * HID:(t - t0 + 1) * HID], pt0[:])
                nc.sync.dma_start(
                    ag_in[0][t0 * 128:(t0 + nb) * 128, :].rearrange("(t p) f -> p t f", p=128),
                    st[:, :nb * HID].rearrange("p (t f) -> p t f", f=HID),
                )
            allgather(0)

            # ---- layer 1: aggregate projected x, then bias+relu ----
            sb1 = make_store(None, b1_t, mybir.ActivationFunctionType.Relu, ag_in[1][:], HID)
            aggregate_project(0, lambda s: nw_d[s], snw_t, sb1)
            allgather(1)

            # ---- layer 2: aggregate h1, then W2 + bias ----
            sb2 = make_store([w2_t], [b2_t], None, ag_in[2][:], HID)
            aggregate_project(1, lambda s: nw_d[s], snw_t, sb2)
            allgather(2)

            # ---- layer 3: aggregate h2; mu/ls projections ----
            sb3 = make_store([wm_t, wl_t], [bm_t, bl_t], None, out_d[:], 2 * OUT)
            aggregate_project(2, lambda s: n1_d[s], sn1_t, sb3)

    # Tile round-robins Pool-DMA completion sems over 8 DMASW lanes without
    # queue awareness, but each sem is hardware-locked to the first SWDGE
    # queue that increments it. Rewrite each gather's queue to lane % 4 so
    # every lane's sem is only ever incremented from one queue.
    for fn in nc.m.functions:
        for blk in fn.blocks:
            for ins in blk.instructions:
                if isinstance(ins, mybir.InstDMAGatherAnt) and ins.sync_info:
                    for u in ins.sync_info.on_update:
                        name = getattr(u, "ant_name", "") or ""
                        if name.startswith("DMASW"):
                            ins.queue_num = int(name[5:].split("_")[0]) % 4
                            break

    nc.compile()
    return nc


def _run(inputs, trace=False):
    x = np.asarray(inputs["x"], np.float32)
    plan, per_core = _prep(
        x, np.asarray(inputs["edge_index"]), np.asarray(inputs["edge_weight"])
    )
    nc = _build(plan)

    x_pad = np.zeros((NPAD, FIN), np.float32)
    x_pad[:N] = x
    R = plan["R"]
    iotaf = np.tile(np.arange(128, dtype=np.float16)[None, :], (128, CPG)).reshape(128, CPG * 128)
    iotar = np.tile(np.arange(128, dtype=np.float16)[None, :], (128, R)).reshape(128, R * 128)
    iotar += np.repeat(np.arange(R, dtype=np.float16) * 128, 128)[None, :]
    shared = {
        "W1": np.asarray(inputs["W1"], np.float32),
        "W2": np.asarray(inputs["W2"], np.float32),
        "Wmu": np.asarray(inputs["Wmu"], np.float32),
        "Wls": np.asarray(inputs["Wls"], np.float32),
        "b1": np.asarray(inputs["b1"], np.float32).reshape(HID, 1),
        "b2": np.asarray(inputs["b2"], np.float32).reshape(HID, 1),
        "bmu": np.asarray(inputs["bmu"], np.float32).reshape(OUT, 1),
        "bls": np.asarray(inputs["bls"], np.float32).reshape(OUT, 1),
        "iotaf": iotaf,
        "iotar": iotar,
        "ident": np.eye(128, dtype=np.float32),
        "identh": np.eye(128, dtype=np.float16),
    }
    in_maps = []
    for d in range(NCORES):
        srcw, dlocv, nwv, n1v, snwv, sn1v = per_core[d]
        m = dict(shared)
        m["xs"] = x_pad[d * SH:(d + 1) * SH]
        m["snw"] = snwv
        m["sn1"] = sn1v
        for s in range(NSUB):
            m[f"srcw{s}"] = srcw[s]
            m[f"dloc{s}"] = dlocv[s]
            m[f"nw{s}"] = nwv[s]
            m[f"n1{s}"] = n1v[s]
        in_maps.append(m)

    res = run_bass_kernel_spmd(nc, in_maps, core_ids=list(range(NCORES)), trace=trace)
    full = np.concatenate([res.results[d]["out"] for d in range(NCORES)], axis=0)
    mu = full[:N, :OUT].copy()
    logstd = full[:N, OUT:].copy()
    return (mu, logstd), res


def kernel(**inputs):
    (mu, logstd), _ = _run(inputs, trace=False)
    return mu, logstd
